# revision 28
# baseline (speedup 1.0000x reference)
"""Trainium2 Bass kernel: 34-channel per-channel GRU (input_size=1) over ragged
sequences + concat features -> linear proj -> BatchNorm(train fwd) -> ReLU ->
linear -> sigmoid.

Strategy:
  - Channel-parallel across the 8 NeuronCores: C=34 padded to 40, 5 channels
    per core, full batch B=256 everywhere. Params replicated per-slice.
  - Batch is sorted by lens (descending) on the host; at step t only the
    active prefix n_t = #{lens > t} of columns is computed. This implements
    the ragged freeze exactly (no masking) and halves the work.
  - Per channel, one [66,128] matmul produces the [z|r] gate preacts and one
    produces [nx|nh] ([gx_n part | Whh_n h part]); the contraction rows are
    [h (64); x_t (1); ones (1)] so the input contribution and both biases ride
    in the same matmul. The z-block weights are pre-negated so a plain sigmoid
    yields zbar = 1-z with no per-partition scale operand.
  - Gate math on ScalarE (sigmoid/tanh) + VectorE; the `nx + r*nh` add is done
    by an identity-matmul PSUM-accumulate on the TensorEngine.
  - The h update uses h' = zbar*n - w3 with w3 = (zbar-1)*h computed OFF the
    critical chain (overlapped with the id-matmul + tanh), so only two
    elementwise ops sit between tanh and the next step's matmuls. Group 1's
    w3 pair runs on GPSIMD; x_t DMAs issue from the idle Sync queue. Per-step
    work is emitted in phases (both groups' matmul/sigmoid/t1 before any
    tanh-tail) to avoid head-of-line blocking in the in-order engine queues.
  - Final features -> partial projection per core -> AllReduce -> BatchNorm
    (batch stats; proj bias cancels in BN) -> ReLU -> pred matvec -> sigmoid.
"""

import sys

sys.path.insert(0, "/opt/trn_rl_repo")

import numpy as np
import ml_dtypes

import concourse.bass as bass
from concourse import bacc, mybir
from concourse.tile import TileContext
from concourse.bass_utils import run_bass_kernel_spmd

B, T, C, H = 256, 512, 34, 64
EPS = 1e-5
NCORES = 8
CPAD = 48 // 48 * 40  # 40 padded channels
CH_LOC = CPAD // NCORES  # 5
GROUPS = [(0, 3), (3, 2)]  # (start_channel, n_channels) per pipeline group

BF16 = mybir.dt.bfloat16
F32 = mybir.dt.float32
bfnp = ml_dtypes.bfloat16
AF = mybir.ActivationFunctionType
OP = mybir.AluOpType


def _v3(ap2, nch, n, p0=None, p1=None, c0=0):
    """[P, CH_LOC*B] AP -> [p0:p1, c0:c0+nch, 0:n] 3D view with 256-col chans."""
    a = ap2.rearrange("p (c b) -> p c b", b=B)
    if p0 is None:
        return a[:, c0 : c0 + nch, 0:n]
    return a[p0:p1, c0 : c0 + nch, 0:n]


def _vc3(ap2, nch, n, p0=None, p1=None):
    """compact work tile [P, W] -> [p, nch, n] over the first nch*n cols."""
    a = ap2[:, 0 : nch * n].rearrange("p (c b) -> p c b", c=nch)
    if p0 is None:
        return a
    return a[p0:p1]


def build_program(t_eff, nsched, dbg=False):
    nc = bacc.Bacc(
        "TRN2", target_bir_lowering=False, debug=False, num_devices=NCORES
    )
    xrow_d = nc.dram_tensor("xrow", [t_eff, CH_LOC * B], BF16, kind="ExternalInput").ap()
    wrz_d = nc.dram_tensor("wrz", [66, CH_LOC * 128], BF16, kind="ExternalInput").ap()
    wnhx_d = nc.dram_tensor("wnhx", [66, CH_LOC * 128], BF16, kind="ExternalInput").ap()
    wp_d = nc.dram_tensor("wp", [64, CH_LOC * 64], BF16, kind="ExternalInput").ap()
    wpred_d = nc.dram_tensor("wpred", [64, 1], BF16, kind="ExternalInput").ap()
    ident_d = nc.dram_tensor("ident", [64, 64], BF16, kind="ExternalInput").ap()
    gamma_d = nc.dram_tensor("gammacol", [64, 1], F32, kind="ExternalInput").ap()
    beta_d = nc.dram_tensor("betacol", [64, 1], F32, kind="ExternalInput").ap()
    bpred_d = nc.dram_tensor("bpredcol", [1, 1], F32, kind="ExternalInput").ap()
    out_d = nc.dram_tensor("out", [1, B], F32, kind="ExternalOutput").ap()
    hdbg_d = (
        nc.dram_tensor("hdbg", [66, CH_LOC * B], F32, kind="ExternalOutput").ap()
        if dbg
        else None
    )
    ones_d = nc.dram_tensor("ones", [2, CH_LOC * B], BF16, kind="ExternalInput").ap()
    cc_in = nc.dram_tensor("cc_in", [64, B], F32).ap()
    cc_out = nc.dram_tensor("cc_out", [64, B], F32, addr_space="Shared").ap()

    with TileContext(nc) as tc:
        with (
            tc.tile_pool(name="const", bufs=1) as cp,
            tc.tile_pool(name="work", bufs=4) as wk,
            tc.tile_pool(name="psum", bufs=1, space="PSUM") as pp,
        ):
            wrz = cp.tile([66, CH_LOC * 128], BF16)
            nc.gpsimd.dma_start(wrz[:], wrz_d[:])
            wnhx = cp.tile([66, CH_LOC * 128], BF16)
            nc.gpsimd.dma_start(wnhx[:], wnhx_d[:])
            ident = cp.tile([64, 64], BF16)
            nc.gpsimd.dma_start(ident[:], ident_d[:])

            # state tile: rows 0:64 = h (bf16), row 64 = x_t, row 65 = ones
            rhs = cp.tile([66, CH_LOC * B], BF16)
            nc.vector.memset(rhs[0:64, :], 0.0)
            nc.gpsimd.dma_start(rhs[64:66, :], ones_d[:])

            for t in range(t_eff):
                n = int(nsched[t])
                st = {}
                # ---- phase 1 (both groups): x-DMA, gate matmuls, sigmoid,
                # t1, the off-chain h-combination term, and the id-matmul.
                # Emitting phase 1 for BOTH groups before any phase 2 keeps
                # group 1's t1 ahead of group 0's tail ops in the in-order
                # DVE queue (no head-of-line blocking on tanh0).
                for (c0, ng) in GROUPS:
                    gi = 0 if c0 == 0 else 1
                    # per-group x_t stream via the idle Sync queue (keeps the
                    # two group pipelines decoupled: no WAR against the other
                    # group's matmuls)
                    nc.sync.dma_start(
                        _v3(rhs, ng, n, 64, 65, c0=c0),
                        xrow_d[t : t + 1, :].rearrange("o (c b) -> o c b", b=B)[
                            :, c0 : c0 + ng, 0:n
                        ],
                    )
                    arz = pp.tile([128, ng * B], F32, tag=f"arz{gi}")
                    anhx = pp.tile([128, ng * B], F32, tag=f"anhx{gi}")
                    for j, c in enumerate(range(c0, c0 + ng)):
                        nc.tensor.matmul(
                            arz[:, j * B : j * B + n],
                            wrz[:, c * 128 : (c + 1) * 128],
                            rhs[:, c * B : c * B + n],
                            start=True,
                            stop=True,
                        )
                    for j, c in enumerate(range(c0, c0 + ng)):
                        # start=True clears has_written for the WHOLE psum
                        # bank; banks hold channel pairs (2k, 2k+1), so only
                        # the even slot may clear or the odd slot's later
                        # id-matmul accumulate would overwrite.
                        nc.tensor.matmul(
                            anhx[:, j * B : j * B + n],
                            wnhx[:, c * 128 : (c + 1) * 128],
                            rhs[:, c * B : c * B + n],
                            start=(j % 2 == 0),
                            stop=False,
                            skip_group_check=True,
                        )
                    # z-block of wrz is pre-negated on the host, so plain
                    # sigmoid gives rows 0:64 = zbar = sig(-Az), rows
                    # 64:128 = r = sig(Ar) with no per-partition scale operand.
                    rz = wk.tile([128, 768], BF16, tag=f"rz{gi}")
                    nc.scalar.activation(
                        _vc3(rz, ng, n),
                        _v3(arz, ng, n),
                        AF.Sigmoid,
                    )
                    # t1 = r * nh   (nh lives on psum partitions 64:128)
                    t1 = wk.tile([64, 768], BF16, tag=f"t1{gi}")
                    nc.vector.tensor_mul(
                        _vc3(t1, ng, n),
                        _vc3(rz, ng, n, 64, 128),
                        _v3(anhx, ng, n, 64, 128),
                    )
                    st[f"rz{gi}"] = rz
                    st[f"anhx{gi}"] = anhx
                    st[f"t1{gi}"] = t1
                # ---- phase 1b (both groups): off-chain h-combination term
                # and the id-matmul. The w3 ops are emitted AFTER both
                # groups' t1 so the slow stt never head-blocks t1 of the
                # other group in the in-order DVE queue:
                #   group 0 (DVE):    w3 = (zbar-1)*h; h' = p - w3
                #   group 1 (GPSIMD): w3 = zbar*h, y = h - w3; h' = p + y
                for (c0, ng) in GROUPS:
                    gi = 0 if c0 == 0 else 1
                    rz, anhx, t1 = st[f"rz{gi}"], st[f"anhx{gi}"], st[f"t1{gi}"]
                    # w3 = zbar*h on GPSIMD for both groups; group 0's
                    # y = h - w3 runs early on the DVE (behind both t1s,
                    # hidden under idmm+tanh), group 1's in phase 2.
                    w3 = wk.tile([64, 768], BF16, tag=f"w3{gi}")
                    nc.gpsimd.tensor_mul(
                        _vc3(w3, ng, n),
                        _vc3(rz, ng, n, 0, 64),
                        _v3(rhs, ng, n, 0, 64, c0=c0),
                    )
                    # nx += I @ t1  (PE accumulate into the nx psum slot)
                    for j in range(ng):
                        nc.tensor.matmul(
                            anhx[0:64, j * B : j * B + n],
                            ident[0:64, :],
                            t1[0:64, j * n : (j + 1) * n],
                            start=False,
                            stop=True,
                            skip_group_check=True,
                        )
                    st[f"w3{gi}"] = w3
                y0 = wk.tile([64, 768], BF16, tag="y0")
                nc.vector.tensor_sub(
                    _vc3(y0, GROUPS[0][1], n),
                    _v3(rhs, GROUPS[0][1], n, 0, 64, c0=GROUPS[0][0]),
                    _vc3(st["w30"], GROUPS[0][1], n),
                )
                st["y0"] = y0
                # ---- phase 2 (both groups): tanh and the 2-op h update.
                for (c0, ng) in GROUPS:
                    gi = 0 if c0 == 0 else 1
                    rz, anhx, w3 = st[f"rz{gi}"], st[f"anhx{gi}"], st[f"w3{gi}"]
                    nh = wk.tile([64, 768], BF16, tag=f"nh{gi}")
                    nc.scalar.activation(
                        _vc3(nh, ng, n), _v3(anhx, ng, n, 0, 64), AF.Tanh
                    )
                    if gi == 1:
                        # y = h - w3, emitted after group 0's tail on the DVE
                        y = wk.tile([64, 768], BF16, tag=f"y{gi}")
                        nc.vector.tensor_sub(
                            _vc3(y, ng, n),
                            _v3(rhs, ng, n, 0, 64, c0=c0),
                            _vc3(w3, ng, n),
                        )
                        st[f"y{gi}"] = y
                    p = wk.tile([64, 768], BF16, tag=f"p{gi}")
                    nc.vector.tensor_mul(
                        _vc3(p, ng, n), _vc3(rz, ng, n, 0, 64), _vc3(nh, ng, n)
                    )
                    y = st[f"y{gi}"]
                    if gi == 0:
                        # h update split by channel: writing channel c0 first
                        # lets the next step's first rz matmul (which only
                        # reads c0's columns) start while ch c0+1.. finish.
                        nc.vector.tensor_add(
                            _v3(rhs, 1, n, 0, 64, c0=c0),
                            _vc3(p, ng, n)[:, 0:1, :],
                            _vc3(y, ng, n)[:, 0:1, :],
                        )
                        nc.vector.tensor_add(
                            _v3(rhs, ng - 1, n, 0, 64, c0=c0 + 1),
                            _vc3(p, ng, n)[:, 1:ng, :],
                            _vc3(y, ng, n)[:, 1:ng, :],
                        )
                    else:
                        nc.vector.tensor_add(
                            _v3(rhs, ng, n, 0, 64, c0=c0),
                            _vc3(p, ng, n),
                            _vc3(y, ng, n),
                        )

            if dbg:
                hf32 = cp.tile([66, CH_LOC * B], F32)
                nc.vector.tensor_copy(hf32[:], rhs[:])
                nc.gpsimd.dma_start(hdbg_d[:], hf32[:])
            # ---- tail: proj partial -> allreduce -> BN -> relu -> pred ----
            wp = cp.tile([64, CH_LOC * 64], BF16)
            nc.gpsimd.dma_start(wp[:], wp_d[:])
            pj = pp.tile([64, B], F32, tag="pproj")
            for c in range(CH_LOC):
                nc.tensor.matmul(
                    pj[:, :],
                    wp[:, c * 64 : (c + 1) * 64],
                    rhs[0:64, c * B : (c + 1) * B],
                    start=(c == 0),
                    stop=(c == CH_LOC - 1),
                )
            pjs = cp.tile([64, B], F32)
            nc.scalar.copy(pjs[:], pj[:])
            nc.gpsimd.dma_start(cc_in[:], pjs[:])
            nc.gpsimd.collective_compute(
                "AllReduce",
                OP.add,
                replica_groups=[list(range(NCORES))],
                ins=[cc_in[:]],
                outs=[cc_out[:]],
            )
            prj = cp.tile([64, B], F32)
            nc.gpsimd.dma_start(prj[:], cc_out[:])

            musum = cp.tile([64, 1], F32)
            nc.vector.tensor_reduce(musum[:], prj[:], mybir.AxisListType.X, OP.add)
            mu = cp.tile([64, 1], F32)
            nc.scalar.mul(mu[:], musum[:], 1.0 / B)
            cen = cp.tile([64, B], F32)
            nc.vector.tensor_scalar_sub(cen[:], prj[:], mu[:, 0:1])
            sq = cp.tile([64, B], F32)
            nc.vector.tensor_mul(sq[:], cen[:], cen[:])
            vsum = cp.tile([64, 1], F32)
            nc.vector.tensor_reduce(vsum[:], sq[:], mybir.AxisListType.X, OP.add)
            v = cp.tile([64, 1], F32)
            nc.scalar.mul(v[:], vsum[:], 1.0 / B)
            veps = cp.tile([64, 1], F32)
            nc.vector.tensor_scalar_add(veps[:], v[:], EPS)
            std = cp.tile([64, 1], F32)
            nc.scalar.activation(std[:], veps[:], AF.Sqrt)
            rstd = cp.tile([64, 1], F32)
            nc.vector.reciprocal(rstd[:], std[:])
            gam = cp.tile([64, 1], F32)
            nc.gpsimd.dma_start(gam[:], gamma_d[:])
            bet = cp.tile([64, 1], F32)
            nc.gpsimd.dma_start(bet[:], beta_d[:])
            sc2 = cp.tile([64, 1], F32)
            nc.vector.tensor_mul(sc2[:], rstd[:], gam[:])
            y = cp.tile([64, B], BF16)
            nc.vector.tensor_scalar(
                y[:], cen[:], sc2[:, 0:1], bet[:, 0:1], OP.mult, OP.add
            )
            yr = cp.tile([64, B], BF16)
            nc.vector.tensor_scalar_max(yr[:], y[:], 0.0)
            wpred = cp.tile([64, 1], BF16)
            nc.gpsimd.dma_start(wpred[:], wpred_d[:])
            pps = pp.tile([1, B], F32, tag="pred")
            nc.tensor.matmul(pps[:], wpred[:, 0:1], yr[:, :], start=True, stop=True)
            bp = cp.tile([1, 1], F32)
            nc.gpsimd.dma_start(bp[:], bpred_d[:])
            osb = cp.tile([1, B], F32)
            nc.scalar.activation(osb[:], pps[:], AF.Sigmoid, bias=bp[0:1, 0:1])
            nc.gpsimd.dma_start(out_d[:], osb[:])

    nc.compile()
    return nc


def prepare_inputs(x, Wih, Whh, b_ih, b_hh, Wp, bp, gamma, beta, Wpred, bpred, lens):
    """Host-side: sort batch by lens desc, pack per-core tensors."""
    x = np.asarray(x)
    lens = np.asarray(lens)
    perm = np.argsort(-lens, kind="stable")
    lens_s = lens[perm]
    x_s = x[perm]  # [B, T, C]

    nsched = np.array([(lens_s > t).sum() for t in range(T)], dtype=np.int64)
    t_eff = int((nsched > 0).sum())
    nsched = nsched[:t_eff]

    # padded params
    WihP = np.zeros((CPAD, 3 * H), np.float32)
    WihP[:C] = np.asarray(Wih)
    WhhP = np.zeros((CPAD, 3 * H, H), np.float32)
    WhhP[:C] = np.asarray(Whh)
    bihP = np.zeros((CPAD, 3 * H), np.float32)
    bihP[:C] = np.asarray(b_ih)
    bhhP = np.zeros((CPAD, 3 * H), np.float32)
    bhhP[:C] = np.asarray(b_hh)
    WpP = np.zeros((H, CPAD * H), np.float32)
    WpP[:, : C * H] = np.asarray(Wp)

    ident = np.eye(64, dtype=bfnp)

    in_maps = []
    for k in range(NCORES):
        gs = list(range(k * CH_LOC, (k + 1) * CH_LOC))
        xrow = np.zeros((t_eff, CH_LOC, B), np.float32)
        wrz = np.zeros((66, CH_LOC * 128), np.float32)
        wnhx = np.zeros((66, CH_LOC * 128), np.float32)
        wp_t = np.zeros((64, CH_LOC * 64), np.float32)
        for c, g in enumerate(gs):
            if g < C:
                xrow[:, c, :] = x_s[:, :t_eff, g].T
            o = c * 128
            # z block (cols 0:64, negated so sigmoid yields zbar directly),
            # r block (cols 64:128)
            wrz[0:64, o : o + 64] = -WhhP[g, H : 2 * H, :].T
            wrz[64, o : o + 64] = -WihP[g, H : 2 * H]
            wrz[65, o : o + 64] = -(bihP[g, H : 2 * H] + bhhP[g, H : 2 * H])
            wrz[0:64, o + 64 : o + 128] = WhhP[g, 0:H, :].T
            wrz[64, o + 64 : o + 128] = WihP[g, 0:H]
            wrz[65, o + 64 : o + 128] = bihP[g, 0:H] + bhhP[g, 0:H]
            # nx block (cols 0:64), nh block (cols 64:128)
            wnhx[64, o : o + 64] = WihP[g, 2 * H : 3 * H]
            wnhx[65, o : o + 64] = bihP[g, 2 * H : 3 * H]
            wnhx[0:64, o + 64 : o + 128] = WhhP[g, 2 * H : 3 * H, :].T
            wnhx[65, o + 64 : o + 128] = bhhP[g, 2 * H : 3 * H]
            wp_t[:, c * 64 : (c + 1) * 64] = WpP[:, g * H : (g + 1) * H].T
        in_maps.append(
            {
                "xrow": xrow.reshape(t_eff, CH_LOC * B).astype(bfnp),
                "wrz": wrz.astype(bfnp),
                "wnhx": wnhx.astype(bfnp),
                "wp": wp_t.astype(bfnp),
                "wpred": np.asarray(Wpred, np.float32).reshape(1, 64).T.astype(bfnp),
                "ident": ident,
                "ones": np.ones((2, CH_LOC * B), bfnp),
                "gammacol": np.asarray(gamma, np.float32).reshape(64, 1),
                "betacol": np.asarray(beta, np.float32).reshape(64, 1),
                "bpredcol": np.asarray(bpred, np.float32).reshape(1, 1),
            }
        )
    return in_maps, perm, t_eff, nsched


_CACHE = {}


def run(inputs, trace=False, dbg=False):
    in_maps, perm, t_eff, nsched = prepare_inputs(**inputs)
    key = (t_eff, tuple(int(v) for v in nsched), dbg)
    if key not in _CACHE:
        _CACHE[key] = build_program(t_eff, nsched, dbg=dbg)
    nc = _CACHE[key]
    res = run_bass_kernel_spmd(
        nc, in_maps, list(range(NCORES)), trace=trace
    )
    out_sorted = np.asarray(res.results[0]["out"][0], np.float32)
    out = np.empty(B, np.float32)
    out[perm] = out_sorted
    return out, res


def kernel(**inputs):
    out, _ = run(inputs, trace=False)
    return out



# revision 30
# speedup vs baseline: 1.0282x; 1.0282x over previous
"""Trainium2 Bass kernel: 34-channel per-channel GRU (input_size=1) over ragged
sequences + concat features -> linear proj -> BatchNorm(train fwd) -> ReLU ->
linear -> sigmoid.

Strategy:
  - Channel-parallel across the 8 NeuronCores: C=34 padded to 40, 5 channels
    per core, full batch B=256 everywhere. Params replicated per-slice.
  - Batch is sorted by lens (descending) on the host; at step t only the
    active prefix n_t = #{lens > t} of columns is computed. This implements
    the ragged freeze exactly (no masking) and halves the work.
  - Per channel, one [66,128] matmul produces the [z|r] gate preacts and one
    produces [nx|nh] ([gx_n part | Whh_n h part]); the contraction rows are
    [h (64); x_t (1); ones (1)] so the input contribution and both biases ride
    in the same matmul. The z-block weights are pre-negated so a plain sigmoid
    yields zbar = 1-z with no per-partition scale operand.
  - Gate math on ScalarE (sigmoid/tanh) + VectorE; the `nx + r*nh` add is done
    by an identity-matmul PSUM-accumulate on the TensorEngine.
  - The h update uses h' = zbar*n - w3 with w3 = (zbar-1)*h computed OFF the
    critical chain (overlapped with the id-matmul + tanh), so only two
    elementwise ops sit between tanh and the next step's matmuls. Group 1's
    w3 pair runs on GPSIMD; x_t DMAs issue from the idle Sync queue. Per-step
    work is emitted in phases (both groups' matmul/sigmoid/t1 before any
    tanh-tail) to avoid head-of-line blocking in the in-order engine queues.
  - Final features -> partial projection per core -> AllReduce -> BatchNorm
    (batch stats; proj bias cancels in BN) -> ReLU -> pred matvec -> sigmoid.
"""

import sys

sys.path.insert(0, "/opt/trn_rl_repo")

import numpy as np
import ml_dtypes

import concourse.bass as bass
from concourse import bacc, mybir
from concourse.tile import TileContext
from concourse.bass_utils import run_bass_kernel_spmd

B, T, C, H = 256, 512, 34, 64
EPS = 1e-5
NCORES = 8
CPAD = 48 // 48 * 40  # 40 padded channels
CH_LOC = CPAD // NCORES  # 5
GROUPS = [(0, 3), (3, 2)]  # (start_channel, n_channels) per pipeline group

BF16 = mybir.dt.bfloat16
F32 = mybir.dt.float32
bfnp = ml_dtypes.bfloat16
AF = mybir.ActivationFunctionType
OP = mybir.AluOpType


def _v3(ap2, nch, n, p0=None, p1=None, c0=0):
    """[P, CH_LOC*B] AP -> [p0:p1, c0:c0+nch, 0:n] 3D view with 256-col chans."""
    a = ap2.rearrange("p (c b) -> p c b", b=B)
    if p0 is None:
        return a[:, c0 : c0 + nch, 0:n]
    return a[p0:p1, c0 : c0 + nch, 0:n]


def _vc3(ap2, nch, n, p0=None, p1=None):
    """compact work tile [P, W] -> [p, nch, n] over the first nch*n cols."""
    a = ap2[:, 0 : nch * n].rearrange("p (c b) -> p c b", c=nch)
    if p0 is None:
        return a
    return a[p0:p1]


def build_program(t_eff, nsched, dbg=False):
    nc = bacc.Bacc(
        "TRN2", target_bir_lowering=False, debug=False, num_devices=NCORES
    )
    xrow_d = nc.dram_tensor("xrow", [t_eff, CH_LOC * B], BF16, kind="ExternalInput").ap()
    wrz_d = nc.dram_tensor("wrz", [66, CH_LOC * 128], BF16, kind="ExternalInput").ap()
    wnhx_d = nc.dram_tensor("wnhx", [66, CH_LOC * 128], BF16, kind="ExternalInput").ap()
    wp_d = nc.dram_tensor("wp", [64, CH_LOC * 64], BF16, kind="ExternalInput").ap()
    wpred_d = nc.dram_tensor("wpred", [64, 1], BF16, kind="ExternalInput").ap()
    ident_d = nc.dram_tensor("ident", [64, 64], BF16, kind="ExternalInput").ap()
    gamma_d = nc.dram_tensor("gammacol", [64, 1], F32, kind="ExternalInput").ap()
    beta_d = nc.dram_tensor("betacol", [64, 1], F32, kind="ExternalInput").ap()
    bpred_d = nc.dram_tensor("bpredcol", [1, 1], F32, kind="ExternalInput").ap()
    out_d = nc.dram_tensor("out", [1, B], F32, kind="ExternalOutput").ap()
    hdbg_d = (
        nc.dram_tensor("hdbg", [66, CH_LOC * B], F32, kind="ExternalOutput").ap()
        if dbg
        else None
    )
    ones_d = nc.dram_tensor("ones", [2, CH_LOC * B], BF16, kind="ExternalInput").ap()
    cc_in = nc.dram_tensor("cc_in", [64, B], F32).ap()
    cc_out = nc.dram_tensor("cc_out", [64, B], F32, addr_space="Shared").ap()

    with TileContext(nc) as tc:
        with (
            tc.tile_pool(name="const", bufs=1) as cp,
            tc.tile_pool(name="work", bufs=4) as wk,
            tc.tile_pool(name="psum", bufs=1, space="PSUM") as pp,
        ):
            wrz = cp.tile([66, CH_LOC * 128], BF16)
            nc.gpsimd.dma_start(wrz[:], wrz_d[:])
            wnhx = cp.tile([66, CH_LOC * 128], BF16)
            nc.gpsimd.dma_start(wnhx[:], wnhx_d[:])
            ident = cp.tile([64, 64], BF16)
            nc.gpsimd.dma_start(ident[:], ident_d[:])

            # state tile: rows 0:64 = h (bf16), row 64 = x_t, row 65 = ones
            rhs = cp.tile([66, CH_LOC * B], BF16)
            nc.vector.memset(rhs[0:64, :], 0.0)
            nc.gpsimd.dma_start(rhs[64:66, :], ones_d[:])

            for t in range(t_eff):
                n = int(nsched[t])
                st = {}
                # ---- phase 1 (both groups): x-DMA, gate matmuls, sigmoid,
                # t1, the off-chain h-combination term, and the id-matmul.
                # Emitting phase 1 for BOTH groups before any phase 2 keeps
                # group 1's t1 ahead of group 0's tail ops in the in-order
                # DVE queue (no head-of-line blocking on tanh0).
                for (c0, ng) in GROUPS:
                    gi = 0 if c0 == 0 else 1
                    # per-group x_t stream via the idle Sync queue (keeps the
                    # two group pipelines decoupled: no WAR against the other
                    # group's matmuls)
                    nc.sync.dma_start(
                        _v3(rhs, ng, n, 64, 65, c0=c0),
                        xrow_d[t : t + 1, :].rearrange("o (c b) -> o c b", b=B)[
                            :, c0 : c0 + ng, 0:n
                        ],
                    )
                    arz = pp.tile([128, ng * B], F32, tag=f"arz{gi}")
                    anhx = pp.tile([128, ng * B], F32, tag=f"anhx{gi}")
                    for j, c in enumerate(range(c0, c0 + ng)):
                        nc.tensor.matmul(
                            arz[:, j * B : j * B + n],
                            wrz[:, c * 128 : (c + 1) * 128],
                            rhs[:, c * B : c * B + n],
                            start=True,
                            stop=True,
                        )
                    for j, c in enumerate(range(c0, c0 + ng)):
                        # start=True clears has_written for the WHOLE psum
                        # bank; banks hold channel pairs (2k, 2k+1), so only
                        # the even slot may clear or the odd slot's later
                        # id-matmul accumulate would overwrite.
                        nc.tensor.matmul(
                            anhx[:, j * B : j * B + n],
                            wnhx[:, c * 128 : (c + 1) * 128],
                            rhs[:, c * B : c * B + n],
                            start=(j % 2 == 0),
                            stop=False,
                            skip_group_check=True,
                        )
                    # z-block of wrz is pre-negated on the host, so plain
                    # sigmoid gives rows 0:64 = zbar = sig(-Az), rows
                    # 64:128 = r = sig(Ar) with no per-partition scale operand.
                    rz = wk.tile([128, 768], BF16, tag=f"rz{gi}")
                    nc.scalar.activation(
                        _vc3(rz, ng, n),
                        _v3(arz, ng, n),
                        AF.Sigmoid,
                    )
                    # t1 = r * nh   (nh lives on psum partitions 64:128)
                    t1 = wk.tile([64, 768], BF16, tag=f"t1{gi}")
                    nc.vector.tensor_mul(
                        _vc3(t1, ng, n),
                        _vc3(rz, ng, n, 64, 128),
                        _v3(anhx, ng, n, 64, 128),
                    )
                    st[f"rz{gi}"] = rz
                    st[f"anhx{gi}"] = anhx
                    st[f"t1{gi}"] = t1
                # ---- phase 1b (both groups): off-chain h-combination term
                # and the id-matmul. The w3 ops are emitted AFTER both
                # groups' t1 so the slow stt never head-blocks t1 of the
                # other group in the in-order DVE queue:
                #   group 0 (DVE):    w3 = (zbar-1)*h; h' = p - w3
                #   group 1 (GPSIMD): w3 = zbar*h, y = h - w3; h' = p + y
                for (c0, ng) in GROUPS:
                    gi = 0 if c0 == 0 else 1
                    rz, anhx, t1 = st[f"rz{gi}"], st[f"anhx{gi}"], st[f"t1{gi}"]
                    w3 = wk.tile([64, 768], BF16, tag=f"w3{gi}")
                    if gi == 0:
                        nc.vector.scalar_tensor_tensor(
                            _vc3(w3, ng, n),
                            _vc3(rz, ng, n, 0, 64),
                            1.0,
                            _v3(rhs, ng, n, 0, 64, c0=c0),
                            OP.subtract,
                            OP.mult,
                        )
                    else:
                        # only w3 here; y = h - w3 runs on DVE in phase 2
                        # (after group 0's tail) so GPSIMD is quiet while
                        # the DVE runs group 0's chain-critical p op.
                        nc.gpsimd.tensor_mul(
                            _vc3(w3, ng, n),
                            _vc3(rz, ng, n, 0, 64),
                            _v3(rhs, ng, n, 0, 64, c0=c0),
                        )
                    # nx += I @ t1  (PE accumulate into the nx psum slot)
                    for j in range(ng):
                        nc.tensor.matmul(
                            anhx[0:64, j * B : j * B + n],
                            ident[0:64, :],
                            t1[0:64, j * n : (j + 1) * n],
                            start=False,
                            stop=True,
                            skip_group_check=True,
                        )
                    st[f"w3{gi}"] = w3
                # ---- phase 2 (both groups): tanh and the 2-op h update.
                for (c0, ng) in GROUPS:
                    gi = 0 if c0 == 0 else 1
                    rz, anhx, w3 = st[f"rz{gi}"], st[f"anhx{gi}"], st[f"w3{gi}"]
                    nh = wk.tile([64, 768], BF16, tag=f"nh{gi}")
                    nc.scalar.activation(
                        _vc3(nh, ng, n), _v3(anhx, ng, n, 0, 64), AF.Tanh
                    )
                    if gi == 1:
                        # y = h - w3, emitted after group 0's tail on the DVE
                        y = wk.tile([64, 768], BF16, tag=f"y{gi}")
                        nc.vector.tensor_sub(
                            _vc3(y, ng, n),
                            _v3(rhs, ng, n, 0, 64, c0=c0),
                            _vc3(w3, ng, n),
                        )
                        st[f"y{gi}"] = y
                    p = wk.tile([64, 768], BF16, tag=f"p{gi}")
                    nc.vector.tensor_mul(
                        _vc3(p, ng, n), _vc3(rz, ng, n, 0, 64), _vc3(nh, ng, n)
                    )
                    hs = _v3(rhs, ng, n, 0, 64, c0=c0)
                    if gi == 0:
                        nc.vector.tensor_sub(hs, _vc3(p, ng, n), _vc3(w3, ng, n))
                    else:
                        nc.vector.tensor_add(
                            hs, _vc3(p, ng, n), _vc3(st[f"y{gi}"], ng, n)
                        )

            if dbg:
                hf32 = cp.tile([66, CH_LOC * B], F32)
                nc.vector.tensor_copy(hf32[:], rhs[:])
                nc.gpsimd.dma_start(hdbg_d[:], hf32[:])
            # ---- tail: proj partial -> allreduce -> BN -> relu -> pred ----
            wp = cp.tile([64, CH_LOC * 64], BF16)
            nc.gpsimd.dma_start(wp[:], wp_d[:])
            pj = pp.tile([64, B], F32, tag="pproj")
            for c in range(CH_LOC):
                nc.tensor.matmul(
                    pj[:, :],
                    wp[:, c * 64 : (c + 1) * 64],
                    rhs[0:64, c * B : (c + 1) * B],
                    start=(c == 0),
                    stop=(c == CH_LOC - 1),
                )
            pjs = cp.tile([64, B], F32)
            nc.scalar.copy(pjs[:], pj[:])
            nc.gpsimd.dma_start(cc_in[:], pjs[:])
            nc.gpsimd.collective_compute(
                "AllReduce",
                OP.add,
                replica_groups=[list(range(NCORES))],
                ins=[cc_in[:]],
                outs=[cc_out[:]],
            )
            prj = cp.tile([64, B], F32)
            nc.gpsimd.dma_start(prj[:], cc_out[:])

            musum = cp.tile([64, 1], F32)
            nc.vector.tensor_reduce(musum[:], prj[:], mybir.AxisListType.X, OP.add)
            mu = cp.tile([64, 1], F32)
            nc.scalar.mul(mu[:], musum[:], 1.0 / B)
            cen = cp.tile([64, B], F32)
            nc.vector.tensor_scalar_sub(cen[:], prj[:], mu[:, 0:1])
            sq = cp.tile([64, B], F32)
            nc.vector.tensor_mul(sq[:], cen[:], cen[:])
            vsum = cp.tile([64, 1], F32)
            nc.vector.tensor_reduce(vsum[:], sq[:], mybir.AxisListType.X, OP.add)
            v = cp.tile([64, 1], F32)
            nc.scalar.mul(v[:], vsum[:], 1.0 / B)
            veps = cp.tile([64, 1], F32)
            nc.vector.tensor_scalar_add(veps[:], v[:], EPS)
            std = cp.tile([64, 1], F32)
            nc.scalar.activation(std[:], veps[:], AF.Sqrt)
            rstd = cp.tile([64, 1], F32)
            nc.vector.reciprocal(rstd[:], std[:])
            gam = cp.tile([64, 1], F32)
            nc.gpsimd.dma_start(gam[:], gamma_d[:])
            bet = cp.tile([64, 1], F32)
            nc.gpsimd.dma_start(bet[:], beta_d[:])
            sc2 = cp.tile([64, 1], F32)
            nc.vector.tensor_mul(sc2[:], rstd[:], gam[:])
            y = cp.tile([64, B], BF16)
            nc.vector.tensor_scalar(
                y[:], cen[:], sc2[:, 0:1], bet[:, 0:1], OP.mult, OP.add
            )
            yr = cp.tile([64, B], BF16)
            nc.vector.tensor_scalar_max(yr[:], y[:], 0.0)
            wpred = cp.tile([64, 1], BF16)
            nc.gpsimd.dma_start(wpred[:], wpred_d[:])
            pps = pp.tile([1, B], F32, tag="pred")
            nc.tensor.matmul(pps[:], wpred[:, 0:1], yr[:, :], start=True, stop=True)
            bp = cp.tile([1, 1], F32)
            nc.gpsimd.dma_start(bp[:], bpred_d[:])
            osb = cp.tile([1, B], F32)
            nc.scalar.activation(osb[:], pps[:], AF.Sigmoid, bias=bp[0:1, 0:1])
            nc.gpsimd.dma_start(out_d[:], osb[:])

    nc.compile()
    return nc


def prepare_inputs(x, Wih, Whh, b_ih, b_hh, Wp, bp, gamma, beta, Wpred, bpred, lens):
    """Host-side: sort batch by lens desc, pack per-core tensors."""
    x = np.asarray(x)
    lens = np.asarray(lens)
    perm = np.argsort(-lens, kind="stable")
    lens_s = lens[perm]
    x_s = x[perm]  # [B, T, C]

    nsched = np.array([(lens_s > t).sum() for t in range(T)], dtype=np.int64)
    t_eff = int((nsched > 0).sum())
    nsched = nsched[:t_eff]

    # padded params
    WihP = np.zeros((CPAD, 3 * H), np.float32)
    WihP[:C] = np.asarray(Wih)
    WhhP = np.zeros((CPAD, 3 * H, H), np.float32)
    WhhP[:C] = np.asarray(Whh)
    bihP = np.zeros((CPAD, 3 * H), np.float32)
    bihP[:C] = np.asarray(b_ih)
    bhhP = np.zeros((CPAD, 3 * H), np.float32)
    bhhP[:C] = np.asarray(b_hh)
    WpP = np.zeros((H, CPAD * H), np.float32)
    WpP[:, : C * H] = np.asarray(Wp)

    ident = np.eye(64, dtype=bfnp)

    in_maps = []
    for k in range(NCORES):
        gs = list(range(k * CH_LOC, (k + 1) * CH_LOC))
        xrow = np.zeros((t_eff, CH_LOC, B), np.float32)
        wrz = np.zeros((66, CH_LOC * 128), np.float32)
        wnhx = np.zeros((66, CH_LOC * 128), np.float32)
        wp_t = np.zeros((64, CH_LOC * 64), np.float32)
        for c, g in enumerate(gs):
            if g < C:
                xrow[:, c, :] = x_s[:, :t_eff, g].T
            o = c * 128
            # z block (cols 0:64, negated so sigmoid yields zbar directly),
            # r block (cols 64:128)
            wrz[0:64, o : o + 64] = -WhhP[g, H : 2 * H, :].T
            wrz[64, o : o + 64] = -WihP[g, H : 2 * H]
            wrz[65, o : o + 64] = -(bihP[g, H : 2 * H] + bhhP[g, H : 2 * H])
            wrz[0:64, o + 64 : o + 128] = WhhP[g, 0:H, :].T
            wrz[64, o + 64 : o + 128] = WihP[g, 0:H]
            wrz[65, o + 64 : o + 128] = bihP[g, 0:H] + bhhP[g, 0:H]
            # nx block (cols 0:64), nh block (cols 64:128)
            wnhx[64, o : o + 64] = WihP[g, 2 * H : 3 * H]
            wnhx[65, o : o + 64] = bihP[g, 2 * H : 3 * H]
            wnhx[0:64, o + 64 : o + 128] = WhhP[g, 2 * H : 3 * H, :].T
            wnhx[65, o + 64 : o + 128] = bhhP[g, 2 * H : 3 * H]
            wp_t[:, c * 64 : (c + 1) * 64] = WpP[:, g * H : (g + 1) * H].T
        in_maps.append(
            {
                "xrow": xrow.reshape(t_eff, CH_LOC * B).astype(bfnp),
                "wrz": wrz.astype(bfnp),
                "wnhx": wnhx.astype(bfnp),
                "wp": wp_t.astype(bfnp),
                "wpred": np.asarray(Wpred, np.float32).reshape(1, 64).T.astype(bfnp),
                "ident": ident,
                "ones": np.ones((2, CH_LOC * B), bfnp),
                "gammacol": np.asarray(gamma, np.float32).reshape(64, 1),
                "betacol": np.asarray(beta, np.float32).reshape(64, 1),
                "bpredcol": np.asarray(bpred, np.float32).reshape(1, 1),
            }
        )
    return in_maps, perm, t_eff, nsched


_CACHE = {}


def run(inputs, trace=False, dbg=False):
    in_maps, perm, t_eff, nsched = prepare_inputs(**inputs)
    key = (t_eff, tuple(int(v) for v in nsched), dbg)
    if key not in _CACHE:
        _CACHE[key] = build_program(t_eff, nsched, dbg=dbg)
    nc = _CACHE[key]
    res = run_bass_kernel_spmd(
        nc, in_maps, list(range(NCORES)), trace=trace
    )
    out_sorted = np.asarray(res.results[0]["out"][0], np.float32)
    out = np.empty(B, np.float32)
    out[perm] = out_sorted
    return out, res


def kernel(**inputs):
    out, _ = run(inputs, trace=False)
    return out



# revision 31
# speedup vs baseline: 1.0420x; 1.0134x over previous
"""Trainium2 Bass kernel: 34-channel per-channel GRU (input_size=1) over ragged
sequences + concat features -> linear proj -> BatchNorm(train fwd) -> ReLU ->
linear -> sigmoid.

Strategy:
  - Channel-parallel across the 8 NeuronCores: C=34 padded to 40, 5 channels
    per core, full batch B=256 everywhere. Params replicated per-slice.
  - Batch is sorted by lens (descending) on the host; at step t only the
    active prefix n_t = #{lens > t} of columns is computed. This implements
    the ragged freeze exactly (no masking) and halves the work.
  - Per channel, one [66,128] matmul produces the [z|r] gate preacts and one
    produces [nx|nh] ([gx_n part | Whh_n h part]); the contraction rows are
    [h (64); x_t (1); ones (1)] so the input contribution and both biases ride
    in the same matmul. The z-block weights are pre-negated so a plain sigmoid
    yields zbar = 1-z with no per-partition scale operand.
  - Gate math on ScalarE (sigmoid/tanh) + VectorE; the `nx + r*nh` add is done
    by an identity-matmul PSUM-accumulate on the TensorEngine.
  - The h update uses h' = zbar*n - w3 with w3 = (zbar-1)*h computed OFF the
    critical chain (overlapped with the id-matmul + tanh), so only two
    elementwise ops sit between tanh and the next step's matmuls. Group 1's
    w3 pair runs on GPSIMD; x_t DMAs issue from the idle Sync queue. Per-step
    work is emitted in phases (both groups' matmul/sigmoid/t1 before any
    tanh-tail) to avoid head-of-line blocking in the in-order engine queues.
  - Final features -> partial projection per core -> AllReduce -> BatchNorm
    (batch stats; proj bias cancels in BN) -> ReLU -> pred matvec -> sigmoid.
"""

import sys

sys.path.insert(0, "/opt/trn_rl_repo")

import numpy as np
import ml_dtypes

import concourse.bass as bass
from concourse import bacc, mybir
from concourse.tile import TileContext
from concourse.bass_utils import run_bass_kernel_spmd

B, T, C, H = 256, 512, 34, 64
EPS = 1e-5
NCORES = 8
CPAD = 48 // 48 * 40  # 40 padded channels
CH_LOC = CPAD // NCORES  # 5
GROUPS = [(0, 3), (3, 2)]  # (start_channel, n_channels) per pipeline group

BF16 = mybir.dt.bfloat16
F32 = mybir.dt.float32
bfnp = ml_dtypes.bfloat16
AF = mybir.ActivationFunctionType
OP = mybir.AluOpType


def _v3(ap2, nch, n, p0=None, p1=None, c0=0):
    """[P, CH_LOC*B] AP -> [p0:p1, c0:c0+nch, 0:n] 3D view with 256-col chans."""
    a = ap2.rearrange("p (c b) -> p c b", b=B)
    if p0 is None:
        return a[:, c0 : c0 + nch, 0:n]
    return a[p0:p1, c0 : c0 + nch, 0:n]


def _vc3(ap2, nch, n, p0=None, p1=None):
    """compact work tile [P, W] -> [p, nch, n] over the first nch*n cols."""
    a = ap2[:, 0 : nch * n].rearrange("p (c b) -> p c b", c=nch)
    if p0 is None:
        return a
    return a[p0:p1]


def build_program(t_eff, nsched, dbg=False):
    nc = bacc.Bacc(
        "TRN2", target_bir_lowering=False, debug=False, num_devices=NCORES
    )
    xrow_d = nc.dram_tensor("xrow", [t_eff, CH_LOC * B], BF16, kind="ExternalInput").ap()
    wrz_d = nc.dram_tensor("wrz", [66, CH_LOC * 128], BF16, kind="ExternalInput").ap()
    wnhx_d = nc.dram_tensor("wnhx", [66, CH_LOC * 128], BF16, kind="ExternalInput").ap()
    wp_d = nc.dram_tensor("wp", [64, CH_LOC * 64], BF16, kind="ExternalInput").ap()
    wpred_d = nc.dram_tensor("wpred", [64, 1], BF16, kind="ExternalInput").ap()
    ident_d = nc.dram_tensor("ident", [64, 64], BF16, kind="ExternalInput").ap()
    gamma_d = nc.dram_tensor("gammacol", [64, 1], F32, kind="ExternalInput").ap()
    beta_d = nc.dram_tensor("betacol", [64, 1], F32, kind="ExternalInput").ap()
    bpred_d = nc.dram_tensor("bpredcol", [1, 1], F32, kind="ExternalInput").ap()
    out_d = nc.dram_tensor("out", [1, B], F32, kind="ExternalOutput").ap()
    hdbg_d = (
        nc.dram_tensor("hdbg", [66, CH_LOC * B], F32, kind="ExternalOutput").ap()
        if dbg
        else None
    )
    ones_d = nc.dram_tensor("ones", [2, CH_LOC * B], BF16, kind="ExternalInput").ap()
    cc_in = nc.dram_tensor("cc_in", [64, B], F32).ap()
    cc_out = nc.dram_tensor("cc_out", [64, B], F32, addr_space="Shared").ap()

    with TileContext(nc) as tc:
        with (
            tc.tile_pool(name="const", bufs=1) as cp,
            tc.tile_pool(name="work", bufs=4) as wk,
            tc.tile_pool(name="psum", bufs=1, space="PSUM") as pp,
        ):
            wrz = cp.tile([66, CH_LOC * 128], BF16)
            nc.gpsimd.dma_start(wrz[:], wrz_d[:])
            wnhx = cp.tile([66, CH_LOC * 128], BF16)
            nc.gpsimd.dma_start(wnhx[:], wnhx_d[:])
            ident = cp.tile([64, 64], BF16)
            nc.gpsimd.dma_start(ident[:], ident_d[:])

            # state tile: rows 0:64 = h (bf16), row 64 = x_t, row 65 = ones
            rhs = cp.tile([66, CH_LOC * B], BF16)
            nc.vector.memset(rhs[0:64, :], 0.0)
            nc.gpsimd.dma_start(rhs[64:66, :], ones_d[:])

            for t in range(t_eff):
                n = int(nsched[t])
                st = {}
                # ---- phase 1 (both groups): x-DMA, gate matmuls, sigmoid,
                # t1, the off-chain h-combination term, and the id-matmul.
                # Emitting phase 1 for BOTH groups before any phase 2 keeps
                # group 1's t1 ahead of group 0's tail ops in the in-order
                # DVE queue (no head-of-line blocking on tanh0).
                for (c0, ng) in GROUPS:
                    gi = 0 if c0 == 0 else 1
                    # per-group x_t stream via the idle Sync queue (keeps the
                    # two group pipelines decoupled: no WAR against the other
                    # group's matmuls)
                    nc.sync.dma_start(
                        _v3(rhs, ng, n, 64, 65, c0=c0),
                        xrow_d[t : t + 1, :].rearrange("o (c b) -> o c b", b=B)[
                            :, c0 : c0 + ng, 0:n
                        ],
                    )
                    arz = pp.tile([128, ng * B], F32, tag=f"arz{gi}")
                    anhx = pp.tile([128, ng * B], F32, tag=f"anhx{gi}")
                    for j, c in enumerate(range(c0, c0 + ng)):
                        nc.tensor.matmul(
                            arz[:, j * B : j * B + n],
                            wrz[:, c * 128 : (c + 1) * 128],
                            rhs[:, c * B : c * B + n],
                            start=True,
                            stop=True,
                        )
                    for j, c in enumerate(range(c0, c0 + ng)):
                        # start=True clears has_written for the WHOLE psum
                        # bank; banks hold channel pairs (2k, 2k+1), so only
                        # the even slot may clear or the odd slot's later
                        # id-matmul accumulate would overwrite.
                        nc.tensor.matmul(
                            anhx[:, j * B : j * B + n],
                            wnhx[:, c * 128 : (c + 1) * 128],
                            rhs[:, c * B : c * B + n],
                            start=(j % 2 == 0),
                            stop=False,
                            skip_group_check=True,
                        )
                    # z-block of wrz is pre-negated on the host, so plain
                    # sigmoid gives rows 0:64 = zbar = sig(-Az), rows
                    # 64:128 = r = sig(Ar) with no per-partition scale operand.
                    rz = wk.tile([128, 768], BF16, tag=f"rz{gi}")
                    nc.scalar.activation(
                        _vc3(rz, ng, n),
                        _v3(arz, ng, n),
                        AF.Sigmoid,
                    )
                    # t1 = r * nh   (nh lives on psum partitions 64:128)
                    t1 = wk.tile([64, 768], BF16, tag=f"t1{gi}")
                    nc.vector.tensor_mul(
                        _vc3(t1, ng, n),
                        _vc3(rz, ng, n, 64, 128),
                        _v3(anhx, ng, n, 64, 128),
                    )
                    st[f"rz{gi}"] = rz
                    st[f"anhx{gi}"] = anhx
                    st[f"t1{gi}"] = t1
                # ---- phase 1b (both groups): off-chain h-combination term
                # and the id-matmul. The w3 ops are emitted AFTER both
                # groups' t1 so the slow stt never head-blocks t1 of the
                # other group in the in-order DVE queue:
                #   group 0 (DVE):    w3 = (zbar-1)*h; h' = p - w3
                #   group 1 (GPSIMD): w3 = zbar*h, y = h - w3; h' = p + y
                for (c0, ng) in GROUPS:
                    gi = 0 if c0 == 0 else 1
                    rz, anhx, t1 = st[f"rz{gi}"], st[f"anhx{gi}"], st[f"t1{gi}"]
                    w3 = wk.tile([64, 768], BF16, tag=f"w3{gi}")
                    if gi == 0:
                        nc.vector.scalar_tensor_tensor(
                            _vc3(w3, ng, n),
                            _vc3(rz, ng, n, 0, 64),
                            1.0,
                            _v3(rhs, ng, n, 0, 64, c0=c0),
                            OP.subtract,
                            OP.mult,
                        )
                    else:
                        # only w3 here; y = h - w3 runs on DVE in phase 2
                        # (after group 0's tail) so GPSIMD is quiet while
                        # the DVE runs group 0's chain-critical p op.
                        nc.gpsimd.tensor_mul(
                            _vc3(w3, ng, n),
                            _vc3(rz, ng, n, 0, 64),
                            _v3(rhs, ng, n, 0, 64, c0=c0),
                        )
                    # nx += I @ t1  (PE accumulate into the nx psum slot)
                    for j in range(ng):
                        nc.tensor.matmul(
                            anhx[0:64, j * B : j * B + n],
                            ident[0:64, :],
                            t1[0:64, j * n : (j + 1) * n],
                            start=False,
                            stop=True,
                            skip_group_check=True,
                        )
                    st[f"w3{gi}"] = w3
                # ---- phase 2 (both groups): tanh and the 2-op h update.
                for (c0, ng) in GROUPS:
                    gi = 0 if c0 == 0 else 1
                    rz, anhx, w3 = st[f"rz{gi}"], st[f"anhx{gi}"], st[f"w3{gi}"]
                    nh = wk.tile([64, 768], BF16, tag=f"nh{gi}")
                    nc.scalar.activation(
                        _vc3(nh, ng, n), _v3(anhx, ng, n, 0, 64), AF.Tanh
                    )
                    if gi == 1:
                        # y = h - w3, emitted after group 0's tail on the DVE
                        y = wk.tile([64, 768], BF16, tag=f"y{gi}")
                        nc.vector.tensor_sub(
                            _vc3(y, ng, n),
                            _v3(rhs, ng, n, 0, 64, c0=c0),
                            _vc3(w3, ng, n),
                        )
                        st[f"y{gi}"] = y
                    p = wk.tile([64, 768], BF16, tag=f"p{gi}")
                    nc.vector.tensor_mul(
                        _vc3(p, ng, n), _vc3(rz, ng, n, 0, 64), _vc3(nh, ng, n)
                    )
                    if gi == 0:
                        # h update split by channel: writing ch c0 first lets
                        # the next step's first rz matmul (which reads only
                        # c0's columns) start while the other channels finish.
                        nc.vector.tensor_sub(
                            _v3(rhs, 1, n, 0, 64, c0=c0),
                            _vc3(p, ng, n)[:, 0:1, :],
                            _vc3(w3, ng, n)[:, 0:1, :],
                        )
                        nc.vector.tensor_sub(
                            _v3(rhs, ng - 1, n, 0, 64, c0=c0 + 1),
                            _vc3(p, ng, n)[:, 1:ng, :],
                            _vc3(w3, ng, n)[:, 1:ng, :],
                        )
                    else:
                        nc.vector.tensor_add(
                            _v3(rhs, ng, n, 0, 64, c0=c0),
                            _vc3(p, ng, n),
                            _vc3(st[f"y{gi}"], ng, n),
                        )

            if dbg:
                hf32 = cp.tile([66, CH_LOC * B], F32)
                nc.vector.tensor_copy(hf32[:], rhs[:])
                nc.gpsimd.dma_start(hdbg_d[:], hf32[:])
            # ---- tail: proj partial -> allreduce -> BN -> relu -> pred ----
            wp = cp.tile([64, CH_LOC * 64], BF16)
            nc.gpsimd.dma_start(wp[:], wp_d[:])
            pj = pp.tile([64, B], F32, tag="pproj")
            for c in range(CH_LOC):
                nc.tensor.matmul(
                    pj[:, :],
                    wp[:, c * 64 : (c + 1) * 64],
                    rhs[0:64, c * B : (c + 1) * B],
                    start=(c == 0),
                    stop=(c == CH_LOC - 1),
                )
            pjs = cp.tile([64, B], F32)
            nc.scalar.copy(pjs[:], pj[:])
            nc.gpsimd.dma_start(cc_in[:], pjs[:])
            nc.gpsimd.collective_compute(
                "AllReduce",
                OP.add,
                replica_groups=[list(range(NCORES))],
                ins=[cc_in[:]],
                outs=[cc_out[:]],
            )
            prj = cp.tile([64, B], F32)
            nc.gpsimd.dma_start(prj[:], cc_out[:])

            musum = cp.tile([64, 1], F32)
            nc.vector.tensor_reduce(musum[:], prj[:], mybir.AxisListType.X, OP.add)
            mu = cp.tile([64, 1], F32)
            nc.scalar.mul(mu[:], musum[:], 1.0 / B)
            cen = cp.tile([64, B], F32)
            nc.vector.tensor_scalar_sub(cen[:], prj[:], mu[:, 0:1])
            sq = cp.tile([64, B], F32)
            nc.vector.tensor_mul(sq[:], cen[:], cen[:])
            vsum = cp.tile([64, 1], F32)
            nc.vector.tensor_reduce(vsum[:], sq[:], mybir.AxisListType.X, OP.add)
            v = cp.tile([64, 1], F32)
            nc.scalar.mul(v[:], vsum[:], 1.0 / B)
            veps = cp.tile([64, 1], F32)
            nc.vector.tensor_scalar_add(veps[:], v[:], EPS)
            std = cp.tile([64, 1], F32)
            nc.scalar.activation(std[:], veps[:], AF.Sqrt)
            rstd = cp.tile([64, 1], F32)
            nc.vector.reciprocal(rstd[:], std[:])
            gam = cp.tile([64, 1], F32)
            nc.gpsimd.dma_start(gam[:], gamma_d[:])
            bet = cp.tile([64, 1], F32)
            nc.gpsimd.dma_start(bet[:], beta_d[:])
            sc2 = cp.tile([64, 1], F32)
            nc.vector.tensor_mul(sc2[:], rstd[:], gam[:])
            y = cp.tile([64, B], BF16)
            nc.vector.tensor_scalar(
                y[:], cen[:], sc2[:, 0:1], bet[:, 0:1], OP.mult, OP.add
            )
            yr = cp.tile([64, B], BF16)
            nc.vector.tensor_scalar_max(yr[:], y[:], 0.0)
            wpred = cp.tile([64, 1], BF16)
            nc.gpsimd.dma_start(wpred[:], wpred_d[:])
            pps = pp.tile([1, B], F32, tag="pred")
            nc.tensor.matmul(pps[:], wpred[:, 0:1], yr[:, :], start=True, stop=True)
            bp = cp.tile([1, 1], F32)
            nc.gpsimd.dma_start(bp[:], bpred_d[:])
            osb = cp.tile([1, B], F32)
            nc.scalar.activation(osb[:], pps[:], AF.Sigmoid, bias=bp[0:1, 0:1])
            nc.gpsimd.dma_start(out_d[:], osb[:])

    nc.compile()
    return nc


def prepare_inputs(x, Wih, Whh, b_ih, b_hh, Wp, bp, gamma, beta, Wpred, bpred, lens):
    """Host-side: sort batch by lens desc, pack per-core tensors."""
    x = np.asarray(x)
    lens = np.asarray(lens)
    perm = np.argsort(-lens, kind="stable")
    lens_s = lens[perm]
    x_s = x[perm]  # [B, T, C]

    nsched = np.array([(lens_s > t).sum() for t in range(T)], dtype=np.int64)
    t_eff = int((nsched > 0).sum())
    nsched = nsched[:t_eff]

    # padded params
    WihP = np.zeros((CPAD, 3 * H), np.float32)
    WihP[:C] = np.asarray(Wih)
    WhhP = np.zeros((CPAD, 3 * H, H), np.float32)
    WhhP[:C] = np.asarray(Whh)
    bihP = np.zeros((CPAD, 3 * H), np.float32)
    bihP[:C] = np.asarray(b_ih)
    bhhP = np.zeros((CPAD, 3 * H), np.float32)
    bhhP[:C] = np.asarray(b_hh)
    WpP = np.zeros((H, CPAD * H), np.float32)
    WpP[:, : C * H] = np.asarray(Wp)

    ident = np.eye(64, dtype=bfnp)

    in_maps = []
    for k in range(NCORES):
        gs = list(range(k * CH_LOC, (k + 1) * CH_LOC))
        xrow = np.zeros((t_eff, CH_LOC, B), np.float32)
        wrz = np.zeros((66, CH_LOC * 128), np.float32)
        wnhx = np.zeros((66, CH_LOC * 128), np.float32)
        wp_t = np.zeros((64, CH_LOC * 64), np.float32)
        for c, g in enumerate(gs):
            if g < C:
                xrow[:, c, :] = x_s[:, :t_eff, g].T
            o = c * 128
            # z block (cols 0:64, negated so sigmoid yields zbar directly),
            # r block (cols 64:128)
            wrz[0:64, o : o + 64] = -WhhP[g, H : 2 * H, :].T
            wrz[64, o : o + 64] = -WihP[g, H : 2 * H]
            wrz[65, o : o + 64] = -(bihP[g, H : 2 * H] + bhhP[g, H : 2 * H])
            wrz[0:64, o + 64 : o + 128] = WhhP[g, 0:H, :].T
            wrz[64, o + 64 : o + 128] = WihP[g, 0:H]
            wrz[65, o + 64 : o + 128] = bihP[g, 0:H] + bhhP[g, 0:H]
            # nx block (cols 0:64), nh block (cols 64:128)
            wnhx[64, o : o + 64] = WihP[g, 2 * H : 3 * H]
            wnhx[65, o : o + 64] = bihP[g, 2 * H : 3 * H]
            wnhx[0:64, o + 64 : o + 128] = WhhP[g, 2 * H : 3 * H, :].T
            wnhx[65, o + 64 : o + 128] = bhhP[g, 2 * H : 3 * H]
            wp_t[:, c * 64 : (c + 1) * 64] = WpP[:, g * H : (g + 1) * H].T
        in_maps.append(
            {
                "xrow": xrow.reshape(t_eff, CH_LOC * B).astype(bfnp),
                "wrz": wrz.astype(bfnp),
                "wnhx": wnhx.astype(bfnp),
                "wp": wp_t.astype(bfnp),
                "wpred": np.asarray(Wpred, np.float32).reshape(1, 64).T.astype(bfnp),
                "ident": ident,
                "ones": np.ones((2, CH_LOC * B), bfnp),
                "gammacol": np.asarray(gamma, np.float32).reshape(64, 1),
                "betacol": np.asarray(beta, np.float32).reshape(64, 1),
                "bpredcol": np.asarray(bpred, np.float32).reshape(1, 1),
            }
        )
    return in_maps, perm, t_eff, nsched


_CACHE = {}


def run(inputs, trace=False, dbg=False):
    in_maps, perm, t_eff, nsched = prepare_inputs(**inputs)
    key = (t_eff, tuple(int(v) for v in nsched), dbg)
    if key not in _CACHE:
        _CACHE[key] = build_program(t_eff, nsched, dbg=dbg)
    nc = _CACHE[key]
    res = run_bass_kernel_spmd(
        nc, in_maps, list(range(NCORES)), trace=trace
    )
    out_sorted = np.asarray(res.results[0]["out"][0], np.float32)
    out = np.empty(B, np.float32)
    out[perm] = out_sorted
    return out, res


def kernel(**inputs):
    out, _ = run(inputs, trace=False)
    return out



# revision 32
# speedup vs baseline: 1.0792x; 1.0357x over previous
"""Trainium2 Bass kernel: 34-channel per-channel GRU (input_size=1) over ragged
sequences + concat features -> linear proj -> BatchNorm(train fwd) -> ReLU ->
linear -> sigmoid.

Strategy:
  - Channel-parallel across the 8 NeuronCores: C=34 padded to 40, 5 channels
    per core, full batch B=256 everywhere. Params replicated per-slice.
  - Batch is sorted by lens (descending) on the host; at step t only the
    active prefix n_t = #{lens > t} of columns is computed. This implements
    the ragged freeze exactly (no masking) and halves the work.
  - Per channel, one [66,128] matmul produces the [z|r] gate preacts and one
    produces [nx|nh] ([gx_n part | Whh_n h part]); the contraction rows are
    [h (64); x_t (1); ones (1)] so the input contribution and both biases ride
    in the same matmul. The z-block weights are pre-negated so a plain sigmoid
    yields zbar = 1-z with no per-partition scale operand.
  - Gate math on ScalarE (sigmoid/tanh) + VectorE; the `nx + r*nh` add is done
    by an identity-matmul PSUM-accumulate on the TensorEngine.
  - The h update uses h' = zbar*n - w3 with w3 = (zbar-1)*h computed OFF the
    critical chain (overlapped with the id-matmul + tanh), so only two
    elementwise ops sit between tanh and the next step's matmuls. Group 1's
    w3 pair runs on GPSIMD; x_t DMAs issue from the idle Sync queue. Per-step
    work is emitted in phases (both groups' matmul/sigmoid/t1 before any
    tanh-tail) to avoid head-of-line blocking in the in-order engine queues.
  - Final features -> partial projection per core -> AllReduce -> BatchNorm
    (batch stats; proj bias cancels in BN) -> ReLU -> pred matvec -> sigmoid.
"""

import sys

sys.path.insert(0, "/opt/trn_rl_repo")

import numpy as np
import ml_dtypes

import concourse.bass as bass
from concourse import bacc, mybir
from concourse.tile import TileContext
from concourse.bass_utils import run_bass_kernel_spmd

B, T, C, H = 256, 512, 34, 64
EPS = 1e-5
NCORES = 8
CPAD = 48 // 48 * 40  # 40 padded channels
CH_LOC = CPAD // NCORES  # 5
GROUPS = [(0, 3), (3, 2)]  # (start_channel, n_channels) per pipeline group

BF16 = mybir.dt.bfloat16
F32 = mybir.dt.float32
bfnp = ml_dtypes.bfloat16
AF = mybir.ActivationFunctionType
OP = mybir.AluOpType


def _v3(ap2, nch, n, p0=None, p1=None, c0=0):
    """[P, CH_LOC*B] AP -> [p0:p1, c0:c0+nch, 0:n] 3D view with 256-col chans."""
    a = ap2.rearrange("p (c b) -> p c b", b=B)
    if p0 is None:
        return a[:, c0 : c0 + nch, 0:n]
    return a[p0:p1, c0 : c0 + nch, 0:n]


def _vc3(ap2, nch, n, p0=None, p1=None):
    """compact work tile [P, W] -> [p, nch, n] over the first nch*n cols."""
    a = ap2[:, 0 : nch * n].rearrange("p (c b) -> p c b", c=nch)
    if p0 is None:
        return a
    return a[p0:p1]


def build_program(t_eff, nsched, dbg=False):
    nc = bacc.Bacc(
        "TRN2", target_bir_lowering=False, debug=False, num_devices=NCORES
    )
    xrow_d = nc.dram_tensor("xrow", [t_eff, CH_LOC * B], BF16, kind="ExternalInput").ap()
    wrz_d = nc.dram_tensor("wrz", [66, CH_LOC * 128], BF16, kind="ExternalInput").ap()
    wnhx_d = nc.dram_tensor("wnhx", [66, CH_LOC * 128], BF16, kind="ExternalInput").ap()
    wp_d = nc.dram_tensor("wp", [64, CH_LOC * 64], BF16, kind="ExternalInput").ap()
    wpred_d = nc.dram_tensor("wpred", [64, 1], BF16, kind="ExternalInput").ap()
    ident_d = nc.dram_tensor("ident", [64, 64], BF16, kind="ExternalInput").ap()
    gamma_d = nc.dram_tensor("gammacol", [64, 1], F32, kind="ExternalInput").ap()
    beta_d = nc.dram_tensor("betacol", [64, 1], F32, kind="ExternalInput").ap()
    bpred_d = nc.dram_tensor("bpredcol", [1, 1], F32, kind="ExternalInput").ap()
    out_d = nc.dram_tensor("out", [1, B], F32, kind="ExternalOutput").ap()
    hdbg_d = (
        nc.dram_tensor("hdbg", [66, CH_LOC * B], F32, kind="ExternalOutput").ap()
        if dbg
        else None
    )
    ones_d = nc.dram_tensor("ones", [2, CH_LOC * B], BF16, kind="ExternalInput").ap()
    cc_in = nc.dram_tensor("cc_in", [64, B], F32).ap()
    cc_out = nc.dram_tensor("cc_out", [64, B], F32, addr_space="Shared").ap()

    with TileContext(nc) as tc:
        with (
            tc.tile_pool(name="const", bufs=1) as cp,
            tc.tile_pool(name="work", bufs=4) as wk,
            tc.tile_pool(name="psum", bufs=1, space="PSUM") as pp,
        ):
            wrz = cp.tile([66, CH_LOC * 128], BF16)
            nc.gpsimd.dma_start(wrz[:], wrz_d[:])
            wnhx = cp.tile([66, CH_LOC * 128], BF16)
            nc.gpsimd.dma_start(wnhx[:], wnhx_d[:])
            ident = cp.tile([64, 64], BF16)
            nc.gpsimd.dma_start(ident[:], ident_d[:])

            # state tile: rows 0:64 = h (bf16), row 64 = x_t, row 65 = ones
            rhs = cp.tile([66, CH_LOC * B], BF16)
            nc.vector.memset(rhs[0:64, :], 0.0)
            nc.gpsimd.dma_start(rhs[64:66, :], ones_d[:])

            for t in range(t_eff):
                n = int(nsched[t])
                st = {}
                # ---- phase 1 (both groups): x-DMA, gate matmuls, sigmoid,
                # t1, the off-chain h-combination term, and the id-matmul.
                # Emitting phase 1 for BOTH groups before any phase 2 keeps
                # group 1's t1 ahead of group 0's tail ops in the in-order
                # DVE queue (no head-of-line blocking on tanh0).
                for (c0, ng) in GROUPS:
                    gi = 0 if c0 == 0 else 1
                    # per-group x_t stream via the idle Sync queue (keeps the
                    # two group pipelines decoupled: no WAR against the other
                    # group's matmuls)
                    nc.sync.dma_start(
                        _v3(rhs, ng, n, 64, 65, c0=c0),
                        xrow_d[t : t + 1, :].rearrange("o (c b) -> o c b", b=B)[
                            :, c0 : c0 + ng, 0:n
                        ],
                    )
                    arz = pp.tile([128, ng * B], F32, tag=f"arz{gi}")
                    anhx = pp.tile([128, ng * B], F32, tag=f"anhx{gi}")
                    for j, c in enumerate(range(c0, c0 + ng)):
                        nc.tensor.matmul(
                            arz[:, j * B : j * B + n],
                            wrz[:, c * 128 : (c + 1) * 128],
                            rhs[:, c * B : c * B + n],
                            start=True,
                            stop=True,
                        )
                    for j, c in enumerate(range(c0, c0 + ng)):
                        # start=True clears has_written for the WHOLE psum
                        # bank; banks hold channel pairs (2k, 2k+1), so only
                        # the even slot may clear or the odd slot's later
                        # id-matmul accumulate would overwrite.
                        nc.tensor.matmul(
                            anhx[:, j * B : j * B + n],
                            wnhx[:, c * 128 : (c + 1) * 128],
                            rhs[:, c * B : c * B + n],
                            start=(j % 2 == 0),
                            stop=False,
                            skip_group_check=True,
                        )
                    # z-block of wrz is pre-negated on the host, so plain
                    # sigmoid gives rows 0:64 = zbar = sig(-Az), rows
                    # 64:128 = r = sig(Ar) with no per-partition scale operand.
                    rz = wk.tile([128, 768], BF16, tag=f"rz{gi}")
                    nc.scalar.activation(
                        _vc3(rz, ng, n),
                        _v3(arz, ng, n),
                        AF.Sigmoid,
                    )
                    # t1 = r * nh   (nh lives on psum partitions 64:128)
                    t1 = wk.tile([64, 768], BF16, tag=f"t1{gi}")
                    nc.vector.tensor_mul(
                        _vc3(t1, ng, n),
                        _vc3(rz, ng, n, 64, 128),
                        _v3(anhx, ng, n, 64, 128),
                    )
                    st[f"rz{gi}"] = rz
                    st[f"anhx{gi}"] = anhx
                    st[f"t1{gi}"] = t1
                # ---- phase 1b (both groups): off-chain h-combination term
                # and the id-matmul. The w3 ops are emitted AFTER both
                # groups' t1 so the slow stt never head-blocks t1 of the
                # other group in the in-order DVE queue:
                #   group 0 (DVE):    w3 = (zbar-1)*h; h' = p - w3
                #   group 1 (GPSIMD): w3 = zbar*h, y = h - w3; h' = p + y
                for (c0, ng) in GROUPS:
                    gi = 0 if c0 == 0 else 1
                    rz, anhx, t1 = st[f"rz{gi}"], st[f"anhx{gi}"], st[f"t1{gi}"]
                    w3 = wk.tile([64, 768], BF16, tag=f"w3{gi}")
                    if gi == 0:
                        nc.vector.scalar_tensor_tensor(
                            _vc3(w3, ng, n),
                            _vc3(rz, ng, n, 0, 64),
                            1.0,
                            _v3(rhs, ng, n, 0, 64, c0=c0),
                            OP.subtract,
                            OP.mult,
                        )
                    else:
                        # only w3 here; y = h - w3 runs on DVE in phase 2
                        # (after group 0's tail) so GPSIMD is quiet while
                        # the DVE runs group 0's chain-critical p op.
                        nc.gpsimd.tensor_mul(
                            _vc3(w3, ng, n),
                            _vc3(rz, ng, n, 0, 64),
                            _v3(rhs, ng, n, 0, 64, c0=c0),
                        )
                    # nx += I @ t1  (PE accumulate into the nx psum slot)
                    for j in range(ng):
                        nc.tensor.matmul(
                            anhx[0:64, j * B : j * B + n],
                            ident[0:64, :],
                            t1[0:64, j * n : (j + 1) * n],
                            start=False,
                            stop=True,
                            skip_group_check=True,
                        )
                    st[f"w3{gi}"] = w3
                # ---- phase 2 (both groups): tanh and the 2-op h update.
                for (c0, ng) in GROUPS:
                    gi = 0 if c0 == 0 else 1
                    rz, anhx, w3 = st[f"rz{gi}"], st[f"anhx{gi}"], st[f"w3{gi}"]
                    nh = wk.tile([64, 768], BF16, tag=f"nh{gi}")
                    nc.scalar.activation(
                        _vc3(nh, ng, n), _v3(anhx, ng, n, 0, 64), AF.Tanh
                    )
                    if gi == 1:
                        # y = h - w3, emitted after group 0's tail on the DVE
                        y = wk.tile([64, 768], BF16, tag=f"y{gi}")
                        nc.vector.tensor_sub(
                            _vc3(y, ng, n),
                            _v3(rhs, ng, n, 0, 64, c0=c0),
                            _vc3(w3, ng, n),
                        )
                        st[f"y{gi}"] = y
                    p = wk.tile([64, 768], BF16, tag=f"p{gi}")
                    if gi == 0:
                        # tail split by channel and interleaved (p0, hs0,
                        # p12, hs12): ch c0's h lands as early as possible so
                        # the next step's first rz matmul (which reads only
                        # c0's columns) starts while the others finish.
                        nc.vector.tensor_mul(
                            _vc3(p, ng, n)[:, 0:1, :],
                            _vc3(rz, ng, n, 0, 64)[:, 0:1, :],
                            _vc3(nh, ng, n)[:, 0:1, :],
                        )
                        nc.vector.tensor_sub(
                            _v3(rhs, 1, n, 0, 64, c0=c0),
                            _vc3(p, ng, n)[:, 0:1, :],
                            _vc3(w3, ng, n)[:, 0:1, :],
                        )
                        nc.vector.tensor_mul(
                            _vc3(p, ng, n)[:, 1:ng, :],
                            _vc3(rz, ng, n, 0, 64)[:, 1:ng, :],
                            _vc3(nh, ng, n)[:, 1:ng, :],
                        )
                        nc.vector.tensor_sub(
                            _v3(rhs, ng - 1, n, 0, 64, c0=c0 + 1),
                            _vc3(p, ng, n)[:, 1:ng, :],
                            _vc3(w3, ng, n)[:, 1:ng, :],
                        )
                    else:
                        nc.vector.tensor_mul(
                            _vc3(p, ng, n),
                            _vc3(rz, ng, n, 0, 64),
                            _vc3(nh, ng, n),
                        )
                        nc.vector.tensor_add(
                            _v3(rhs, ng, n, 0, 64, c0=c0),
                            _vc3(p, ng, n),
                            _vc3(st[f"y{gi}"], ng, n),
                        )

            if dbg:
                hf32 = cp.tile([66, CH_LOC * B], F32)
                nc.vector.tensor_copy(hf32[:], rhs[:])
                nc.gpsimd.dma_start(hdbg_d[:], hf32[:])
            # ---- tail: proj partial -> allreduce -> BN -> relu -> pred ----
            wp = cp.tile([64, CH_LOC * 64], BF16)
            nc.gpsimd.dma_start(wp[:], wp_d[:])
            pj = pp.tile([64, B], F32, tag="pproj")
            for c in range(CH_LOC):
                nc.tensor.matmul(
                    pj[:, :],
                    wp[:, c * 64 : (c + 1) * 64],
                    rhs[0:64, c * B : (c + 1) * B],
                    start=(c == 0),
                    stop=(c == CH_LOC - 1),
                )
            pjs = cp.tile([64, B], F32)
            nc.scalar.copy(pjs[:], pj[:])
            nc.gpsimd.dma_start(cc_in[:], pjs[:])
            nc.gpsimd.collective_compute(
                "AllReduce",
                OP.add,
                replica_groups=[list(range(NCORES))],
                ins=[cc_in[:]],
                outs=[cc_out[:]],
            )
            prj = cp.tile([64, B], F32)
            nc.gpsimd.dma_start(prj[:], cc_out[:])

            musum = cp.tile([64, 1], F32)
            nc.vector.tensor_reduce(musum[:], prj[:], mybir.AxisListType.X, OP.add)
            mu = cp.tile([64, 1], F32)
            nc.scalar.mul(mu[:], musum[:], 1.0 / B)
            cen = cp.tile([64, B], F32)
            nc.vector.tensor_scalar_sub(cen[:], prj[:], mu[:, 0:1])
            sq = cp.tile([64, B], F32)
            nc.vector.tensor_mul(sq[:], cen[:], cen[:])
            vsum = cp.tile([64, 1], F32)
            nc.vector.tensor_reduce(vsum[:], sq[:], mybir.AxisListType.X, OP.add)
            v = cp.tile([64, 1], F32)
            nc.scalar.mul(v[:], vsum[:], 1.0 / B)
            veps = cp.tile([64, 1], F32)
            nc.vector.tensor_scalar_add(veps[:], v[:], EPS)
            std = cp.tile([64, 1], F32)
            nc.scalar.activation(std[:], veps[:], AF.Sqrt)
            rstd = cp.tile([64, 1], F32)
            nc.vector.reciprocal(rstd[:], std[:])
            gam = cp.tile([64, 1], F32)
            nc.gpsimd.dma_start(gam[:], gamma_d[:])
            bet = cp.tile([64, 1], F32)
            nc.gpsimd.dma_start(bet[:], beta_d[:])
            sc2 = cp.tile([64, 1], F32)
            nc.vector.tensor_mul(sc2[:], rstd[:], gam[:])
            y = cp.tile([64, B], BF16)
            nc.vector.tensor_scalar(
                y[:], cen[:], sc2[:, 0:1], bet[:, 0:1], OP.mult, OP.add
            )
            yr = cp.tile([64, B], BF16)
            nc.vector.tensor_scalar_max(yr[:], y[:], 0.0)
            wpred = cp.tile([64, 1], BF16)
            nc.gpsimd.dma_start(wpred[:], wpred_d[:])
            pps = pp.tile([1, B], F32, tag="pred")
            nc.tensor.matmul(pps[:], wpred[:, 0:1], yr[:, :], start=True, stop=True)
            bp = cp.tile([1, 1], F32)
            nc.gpsimd.dma_start(bp[:], bpred_d[:])
            osb = cp.tile([1, B], F32)
            nc.scalar.activation(osb[:], pps[:], AF.Sigmoid, bias=bp[0:1, 0:1])
            nc.gpsimd.dma_start(out_d[:], osb[:])

    nc.compile()
    return nc


def prepare_inputs(x, Wih, Whh, b_ih, b_hh, Wp, bp, gamma, beta, Wpred, bpred, lens):
    """Host-side: sort batch by lens desc, pack per-core tensors."""
    x = np.asarray(x)
    lens = np.asarray(lens)
    perm = np.argsort(-lens, kind="stable")
    lens_s = lens[perm]
    x_s = x[perm]  # [B, T, C]

    nsched = np.array([(lens_s > t).sum() for t in range(T)], dtype=np.int64)
    t_eff = int((nsched > 0).sum())
    nsched = nsched[:t_eff]

    # padded params
    WihP = np.zeros((CPAD, 3 * H), np.float32)
    WihP[:C] = np.asarray(Wih)
    WhhP = np.zeros((CPAD, 3 * H, H), np.float32)
    WhhP[:C] = np.asarray(Whh)
    bihP = np.zeros((CPAD, 3 * H), np.float32)
    bihP[:C] = np.asarray(b_ih)
    bhhP = np.zeros((CPAD, 3 * H), np.float32)
    bhhP[:C] = np.asarray(b_hh)
    WpP = np.zeros((H, CPAD * H), np.float32)
    WpP[:, : C * H] = np.asarray(Wp)

    ident = np.eye(64, dtype=bfnp)

    in_maps = []
    for k in range(NCORES):
        gs = list(range(k * CH_LOC, (k + 1) * CH_LOC))
        xrow = np.zeros((t_eff, CH_LOC, B), np.float32)
        wrz = np.zeros((66, CH_LOC * 128), np.float32)
        wnhx = np.zeros((66, CH_LOC * 128), np.float32)
        wp_t = np.zeros((64, CH_LOC * 64), np.float32)
        for c, g in enumerate(gs):
            if g < C:
                xrow[:, c, :] = x_s[:, :t_eff, g].T
            o = c * 128
            # z block (cols 0:64, negated so sigmoid yields zbar directly),
            # r block (cols 64:128)
            wrz[0:64, o : o + 64] = -WhhP[g, H : 2 * H, :].T
            wrz[64, o : o + 64] = -WihP[g, H : 2 * H]
            wrz[65, o : o + 64] = -(bihP[g, H : 2 * H] + bhhP[g, H : 2 * H])
            wrz[0:64, o + 64 : o + 128] = WhhP[g, 0:H, :].T
            wrz[64, o + 64 : o + 128] = WihP[g, 0:H]
            wrz[65, o + 64 : o + 128] = bihP[g, 0:H] + bhhP[g, 0:H]
            # nx block (cols 0:64), nh block (cols 64:128)
            wnhx[64, o : o + 64] = WihP[g, 2 * H : 3 * H]
            wnhx[65, o : o + 64] = bihP[g, 2 * H : 3 * H]
            wnhx[0:64, o + 64 : o + 128] = WhhP[g, 2 * H : 3 * H, :].T
            wnhx[65, o + 64 : o + 128] = bhhP[g, 2 * H : 3 * H]
            wp_t[:, c * 64 : (c + 1) * 64] = WpP[:, g * H : (g + 1) * H].T
        in_maps.append(
            {
                "xrow": xrow.reshape(t_eff, CH_LOC * B).astype(bfnp),
                "wrz": wrz.astype(bfnp),
                "wnhx": wnhx.astype(bfnp),
                "wp": wp_t.astype(bfnp),
                "wpred": np.asarray(Wpred, np.float32).reshape(1, 64).T.astype(bfnp),
                "ident": ident,
                "ones": np.ones((2, CH_LOC * B), bfnp),
                "gammacol": np.asarray(gamma, np.float32).reshape(64, 1),
                "betacol": np.asarray(beta, np.float32).reshape(64, 1),
                "bpredcol": np.asarray(bpred, np.float32).reshape(1, 1),
            }
        )
    return in_maps, perm, t_eff, nsched


_CACHE = {}


def run(inputs, trace=False, dbg=False):
    in_maps, perm, t_eff, nsched = prepare_inputs(**inputs)
    key = (t_eff, tuple(int(v) for v in nsched), dbg)
    if key not in _CACHE:
        _CACHE[key] = build_program(t_eff, nsched, dbg=dbg)
    nc = _CACHE[key]
    res = run_bass_kernel_spmd(
        nc, in_maps, list(range(NCORES)), trace=trace
    )
    out_sorted = np.asarray(res.results[0]["out"][0], np.float32)
    out = np.empty(B, np.float32)
    out[perm] = out_sorted
    return out, res


def kernel(**inputs):
    out, _ = run(inputs, trace=False)
    return out

